# revision 7
# baseline (speedup 1.0000x reference)
# Trainium2 Bass kernel for nn_LinearNonlinearRelease, v2.
#
# Per-core chunked scan: C=64-step chunks, FSLOT=18 chunk-slots/cell (2304
# chunks on 128 partitions x 252 lanes), warmup = 64 hard-clip steps +
# (16 hard + 48 exact) with two state shifts  [equivalent to the validated
# Wh=80/We=48 window], pow16 smooth clamp (all-DVE, no ACT round trips),
# fp16 scan state, fp16 conv2 matmuls.
#
#   smooth_clamp(x,H) = min(max(x,E), H-E)
#   E = min(exp(min(x,H-x)-1),1) ~= min(((max(min(x,H-x,1),-15)+15)/16)^16, 1)
#
# rp_scan / rel_scan are chunk-major: col = lane*C + s, lane = c*FSLOT+f8,
# so phase-1/phase-3 PE transposes move packed [128,128] blocks directly.
import numpy as np

NUM_CELLS = 14
FREQ = 64
D = 1048576
STEADY = 10 * FREQ            # 640
K0 = 20
K1 = 32
PADDING = STEADY + (K0 - 1) + (K1 - 1)   # 690
NCORES = 8
SP = D // NCORES              # 131072

C = 64                        # chunk length (steps)
FSLOT = 18                    # chunk-slots per cell
NCHK = 128 * FSLOT            # 2304 chunks
FREE = NUM_CELLS * FSLOT      # 252 lanes
G0 = FREE // 2                # group split
FSX = 9                       # fine-col blocks (NJX = 9*128)
NJX = FSX * 128               # 1152 fine cols; t' = 128*col + r
XS_LEN = 128 * NJX            # 147456
EHARD = 0.3679


def _f32(x):
    return np.asarray(x, np.float32)


def _elu_np(x):
    return np.where(x > 0, x, np.expm1(x)).astype(np.float32)


def _smooth_clamp_np(x, high):
    x = _elu_np(np.float32(x) - np.float32(1.0)) + np.float32(1.0)
    x = _elu_np(np.float32(high) - np.float32(1.0) - x) - np.float32(high) + np.float32(1.0)
    return (-x).astype(np.float32)


def _compute_kernel_np(log_kernel_speed, cell_types):
    t = (np.float32(0.3) - np.arange(K0, dtype=np.float32) / np.float32(FREQ))[None, :]
    ks = np.exp(_f32(log_kernel_speed))[:, None].astype(np.float32)
    tau_r = (np.float32(0.05) * ks).astype(np.float32)
    tau_d = (np.float32(0.05) * ks).astype(np.float32)
    phi = (np.float32(-np.pi) * np.float32(0.2 / 1.4) * ks).astype(np.float32)
    kernel = (-(t / tau_r) ** 3 / (1.0 + t / tau_r)
              * np.exp(-((t / tau_d) ** 2))
              * np.cos(2.0 * np.float32(np.pi) * t / phi + np.float32(100.0))).astype(np.float32)
    kernel = kernel / np.linalg.norm(kernel.astype(np.float64), axis=1, keepdims=True).astype(np.float32)
    kernel = (-kernel * _f32(cell_types)[:, None]).astype(np.float32)
    return kernel  # (C, K0)


class _Prog:
    pass


_PROG_CACHE = {}


def build_program(key=0):
    if key in _PROG_CACHE:
        return _PROG_CACHE[key]
    import concourse.bacc as bacc
    import concourse.mybir as mybir
    import concourse.tile as tile

    F32 = mybir.dt.float32
    F16 = mybir.dt.float16
    F32R = mybir.dt.float32r
    Alu = mybir.AluOpType
    Act = mybir.ActivationFunctionType

    nc = bacc.Bacc(None, target_bir_lowering=False)

    xs_e = nc.declare_dram_parameter("xs", [XS_LEN], F16, isOutput=False)
    w1_e = nc.declare_dram_parameter("w1", [NUM_CELLS, 128, 128], F16, isOutput=False)
    w2_e = nc.declare_dram_parameter("w2", [NUM_CELLS, 128, 128], F16, isOutput=False)
    g1_e = nc.declare_dram_parameter("g1", [128, 128], F16, isOutput=False)
    g2_e = nc.declare_dram_parameter("g2", [128, 128], F16, isOutput=False)
    idf32_e = nc.declare_dram_parameter("idf32", [128, 128], F32, isOutput=False)
    idf16_e = nc.declare_dram_parameter("idf16", [128, 128], F16, isOutput=False)
    # scan per-lane constant rows (broadcast to all partitions), fp16:
    # 0 cp12, 1 Hr, 2 Hi, 3 cp01, 4 Hr-EHARD, 5 Hi-EHARD,
    # 6 (Hr+15)/16+15/16, 7 (Hi+15)/16+15/16
    cc_e = nc.declare_dram_parameter("cc", [8, FREE], F16, isOutput=False)
    sg_e = nc.declare_dram_parameter("sg", [2, NUM_CELLS], F32, isOutput=False)
    fn_e = nc.declare_dram_parameter("fn", [2, NUM_CELLS], F32, isOutput=False)
    out_e = nc.declare_dram_parameter("out", [NUM_CELLS, SP], F32, isOutput=True)

    with tile.TileContext(nc) as tc:
        with tc.tile_pool(name="persist", bufs=1) as pp, \
             tc.tile_pool(name="wstage", bufs=3) as wp, \
             tc.tile_pool(name="tmaj", bufs=3) as mp, \
             tc.tile_pool(name="tmp", bufs=2) as sp, \
             tc.tile_pool(name="pconv", bufs=2, space="PSUM") as pcv, \
             tc.tile_pool(name="ptp", bufs=3, space="PSUM") as ptp:

            # ---- phase 0: loads & constants ----
            Xc = pp.tile([128, NJX], F16)
            nc.sync.dma_start(Xc[:], xs_e[:].rearrange("(p j) -> p j", j=NJX))
            idf32 = pp.tile([128, 128], F32)
            idf16 = pp.tile([128, 128], F16)
            nc.sync.dma_start(idf32[:], idf32_e[:])
            nc.sync.dma_start(idf16[:], idf16_e[:])
            g1 = pp.tile([128, 128], F16)
            g2 = pp.tile([128, 128], F16)
            nc.sync.dma_start(g1[:], g1_e[:])
            nc.sync.dma_start(g2[:], g2_e[:])
            CT = []
            for i in range(6):
                t_ = pp.tile([128, FREE], F16, name=f"ct{i}")
                nc.sync.dma_start(t_[:], cc_e[i:i + 1, :].to_broadcast([128, FREE]))
                CT.append(t_)
            CP12T, HRT, HIT, CP01T, HRET, HIET = CT
            SGT = pp.tile([128, NUM_CELLS], F32)
            SBT = pp.tile([128, NUM_CELLS], F32)
            FNT = pp.tile([128, NUM_CELLS], F32)
            FBT = pp.tile([128, NUM_CELLS], F32)
            nc.sync.dma_start(SGT[:], sg_e[0:1, :].to_broadcast([128, NUM_CELLS]))
            nc.sync.dma_start(SBT[:], sg_e[1:2, :].to_broadcast([128, NUM_CELLS]))
            nc.sync.dma_start(FNT[:], fn_e[0:1, :].to_broadcast([128, NUM_CELLS]))
            nc.sync.dma_start(FBT[:], fn_e[1:2, :].to_broadcast([128, NUM_CELLS]))

            # x fine layout: Xf[r, 9p+b] = Xc[p, 128b+r] -> xs[128*(9p+b)+r]
            Xf = pp.tile([128, NJX], F16)
            for b in range(FSX):
                tpp = ptp.tile([128, 128], F16, tag="tp16")
                nc.tensor.transpose(tpp[:], Xc[:, b * 128:(b + 1) * 128], idf16[:])
                nc.vector.tensor_copy(Xf[:, b::FSX], tpp[:])

            rp_scan = pp.tile([128, FREE * C], F16)   # q = 1-rp, chunk-major
            rel_scan = pp.tile([128, FREE * C], F16)

            # ---- phase 1: conv1 + q=1-sigmoid + re-layout ----
            col_blocks = [(0, 512), (512, 512), (1024, 127)]
            for c in range(NUM_CELLS):
                w1t = wp.tile([128, 128], F16, tag="w1t")
                w2t = wp.tile([128, 128], F16, tag="w2t")
                nc.sync.dma_start(w1t[:], w1_e[c])
                nc.sync.dma_start(w2t[:], w2_e[c])

                qpt = mp.tile([128, NJX], F16, tag="qpt")
                nc.gpsimd.memset(qpt[:, NJX - 1:NJX], 0.5)
                for (b0, bn) in col_blocks:
                    ps = pcv.tile([128, 512], F32, tag="pconv")
                    nc.tensor.matmul(ps[:, 0:bn], w1t[:], Xf[:, b0:b0 + bn],
                                     start=True, stop=False)
                    nc.tensor.matmul(ps[:, 0:bn], w2t[:], Xf[:, b0 + 1:b0 + 1 + bn],
                                     start=False, stop=True)
                    # q = sigmoid(-slope*y + slope*off) = 1 - rp
                    nc.scalar.activation(qpt[:, b0:b0 + bn], ps[:, 0:bn], Act.Sigmoid,
                                         bias=SBT[:, c:c + 1], scale=SGT[:, c:c + 1])
                # fine col pp*9+a holds chunks (pp*18+2a, pp*18+2a+1)
                for a in range(FSX):
                    tpp = ptp.tile([128, 128], F16, tag="tp16")
                    nc.tensor.transpose(tpp[:], qpt[:, a::FSX], idf16[:])
                    base = c * FSLOT + 2 * a
                    nc.vector.tensor_copy(rp_scan[:, base * C:(base + 2) * C], tpp[:])

            # ---- phase 2: scan ----
            RR = pp.tile([128, FREE], F16)
            IP = pp.tile([128, FREE], F16)
            RR2 = pp.tile([128, FREE], F16)
            IP2 = pp.tile([128, FREE], F16)
            nc.vector.tensor_copy(RR[:], HRT[:])
            nc.vector.tensor_copy(IP[:], HIT[:])

            GS = [(0, G0), (G0, FREE - G0)]

            def qsl(s, g0, gw):
                return rp_scan[:, g0 * C + s:(g0 + gw) * C:C]

            def hard_step(s, RRt, IPt):
                for (g0, gw) in GS:
                    gs = slice(g0, g0 + gw)
                    q = qsl(s, g0, gw)
                    rr = RRt[:, gs]
                    ip = IPt[:, gs]
                    rl = sp.tile([128, FREE], F16, tag="h_rl", name="h_rl")[:, gs]
                    m1 = sp.tile([128, FREE], F16, tag="h_m1", name="h_m1")[:, gs]
                    xr = sp.tile([128, FREE], F16, tag="h_xr", name="h_xr")[:, gs]
                    pa = sp.tile([128, FREE], F16, tag="h_pa", name="h_pa")[:, gs]
                    pb = sp.tile([128, FREE], F16, tag="h_pb", name="h_pb")[:, gs]
                    w_ = sp.tile([128, FREE], F16, tag="h_w", name="h_w")[:, gs]
                    nc.vector.tensor_tensor(rl, q, rr, Alu.mult)
                    nc.gpsimd.tensor_tensor(pa, rl, CP01T[:, gs], Alu.add)
                    nc.gpsimd.tensor_tensor(m1, ip, CP12T[:, gs], Alu.mult)
                    nc.gpsimd.tensor_tensor(pb, pa, ip, Alu.add)
                    nc.gpsimd.tensor_tensor(xr, m1, rl, Alu.add)
                    nc.vector.tensor_tensor(rr, xr, HRET[:, gs], Alu.min)
                    nc.gpsimd.tensor_tensor(w_, pb, rr, Alu.subtract)
                    nc.vector.tensor_tensor(ip, w_, HIET[:, gs], Alu.min)

            def exact_step(s, RRt, IPt, store):
                for (g0, gw) in GS:
                    gs = slice(g0, g0 + gw)
                    q = qsl(s, g0, gw)
                    rr = RRt[:, gs]
                    ip = IPt[:, gs]
                    rl = sp.tile([128, FREE], F16, tag="e_rl", name="e_rl")[:, gs]
                    m1 = sp.tile([128, FREE], F16, tag="e_m1", name="e_m1")[:, gs]
                    xr = sp.tile([128, FREE], F16, tag="e_xr", name="e_xr")[:, gs]
                    x1 = sp.tile([128, FREE], F16, tag="e_x1", name="e_x1")[:, gs]
                    h1 = sp.tile([128, FREE], F16, tag="e_h1", name="e_h1")[:, gs]
                    ww = sp.tile([128, FREE], F16, tag="e_ww", name="e_ww")[:, gs]
                    E1 = sp.tile([128, FREE], F16, tag="e_E1", name="e_E1")[:, gs]
                    u1 = sp.tile([128, FREE], F16, tag="e_u1", name="e_u1")[:, gs]
                    f1 = sp.tile([128, FREE], F16, tag="e_f1", name="e_f1")[:, gs]
                    pa = sp.tile([128, FREE], F16, tag="e_pa", name="e_pa")[:, gs]
                    pb = sp.tile([128, FREE], F16, tag="e_pb", name="e_pb")[:, gs]
                    w_ = sp.tile([128, FREE], F16, tag="e_w", name="e_w")[:, gs]
                    x2 = sp.tile([128, FREE], F16, tag="e_x2", name="e_x2")[:, gs]
                    h2 = sp.tile([128, FREE], F16, tag="e_h2", name="e_h2")[:, gs]
                    w2 = sp.tile([128, FREE], F16, tag="e_w2", name="e_w2")[:, gs]
                    E2 = sp.tile([128, FREE], F16, tag="e_E2", name="e_E2")[:, gs]
                    u2 = sp.tile([128, FREE], F16, tag="e_u2", name="e_u2")[:, gs]
                    f2 = sp.tile([128, FREE], F16, tag="e_f2", name="e_f2")[:, gs]
                    nc.vector.tensor_tensor(rl, q, rr, Alu.mult)
                    if store:
                        rel_s = rel_scan[:, g0 * C + s:(g0 + gw) * C:C]
                        nc.gpsimd.tensor_tensor(rel_s, rr, rl, Alu.subtract)
                    nc.gpsimd.tensor_tensor(pa, rl, CP01T[:, gs], Alu.add)
                    nc.gpsimd.tensor_tensor(m1, ip, CP12T[:, gs], Alu.mult)
                    nc.gpsimd.tensor_tensor(pb, pa, ip, Alu.add)
                    nc.gpsimd.tensor_tensor(xr, m1, rl, Alu.add)
                    # clamp1: E1 = exp(min(min(xr,Hr-xr)-1, 0)) <= 1
                    nc.gpsimd.tensor_tensor(h1, HRT[:, gs], xr, Alu.subtract)
                    nc.vector.tensor_tensor(ww, xr, h1, Alu.min)
                    nc.vector.tensor_scalar(x1, ww, 1.0, 0.0, Alu.subtract, Alu.min)
                    nc.scalar.activation(E1, x1, Act.Exp, bias=0.0, scale=1.0)
                    nc.vector.tensor_tensor(u1, xr, E1, Alu.max)
                    nc.gpsimd.tensor_tensor(f1, HRT[:, gs], E1, Alu.subtract)
                    nc.vector.tensor_tensor(rr, u1, f1, Alu.min)
                    nc.gpsimd.tensor_tensor(w_, pb, rr, Alu.subtract)
                    # clamp2
                    nc.gpsimd.tensor_tensor(h2, HIT[:, gs], w_, Alu.subtract)
                    nc.vector.tensor_tensor(w2, w_, h2, Alu.min)
                    nc.vector.tensor_scalar(x2, w2, 1.0, 0.0, Alu.subtract, Alu.min)
                    nc.scalar.activation(E2, x2, Act.Exp, bias=0.0, scale=1.0)
                    nc.vector.tensor_tensor(u2, w_, E2, Alu.max)
                    nc.gpsimd.tensor_tensor(f2, HIT[:, gs], E2, Alu.subtract)
                    nc.vector.tensor_tensor(ip, u2, f2, Alu.min)

            def shift_state(RRs, IPs, RRd, IPd):
                # chunk j -> j+1: lane f8+1; partition+1 for f8=0
                nc.vector.tensor_copy(RRd[:], HRT[:])
                nc.vector.tensor_copy(IPd[:], HIT[:])
                src = RRs[:].rearrange("p (c f) -> p c f", f=FSLOT)
                dst = RRd[:].rearrange("p (c f) -> p c f", f=FSLOT)
                nc.vector.tensor_copy(dst[:, :, 1:FSLOT], src[:, :, 0:FSLOT - 1])
                srci = IPs[:].rearrange("p (c f) -> p c f", f=FSLOT)
                dsti = IPd[:].rearrange("p (c f) -> p c f", f=FSLOT)
                nc.vector.tensor_copy(dsti[:, :, 1:FSLOT], srci[:, :, 0:FSLOT - 1])
                nc.sync.dma_start(dst[1:128, :, 0:1], src[0:127, :, FSLOT - 1:FSLOT])
                nc.sync.dma_start(dsti[1:128, :, 0:1], srci[0:127, :, FSLOT - 1:FSLOT])

            for s in range(C):
                hard_step(s, RR, IP)
            shift_state(RR, IP, RR2, IP2)
            for s in range(36):
                hard_step(s, RR2, IP2)
            for s in range(36, C):
                exact_step(s, RR2, IP2, store=False)
            shift_state(RR2, IP2, RR, IP)
            # NOTE: no exact-init injection -- the reference's 640 STEADY steps
            # wash out the t=0 initial condition; warmup converges to the same
            # attractor through the constant pad region.
            for s in range(C):
                exact_step(s, RR, IP, store=True)

            # ---- phase 3: transpose back, conv2 (fp16), affine, out ----
            U = SP // 128           # 1024
            ob_blocks = [(0, 512), (512, 512)]
            for c in range(NUM_CELLS):
                rlt = mp.tile([128, NJX], F16, tag="rlt")
                for a in range(FSX):
                    tpp = ptp.tile([128, 128], F16, tag="tp16")
                    base = c * FSLOT + 2 * a
                    nc.tensor.transpose(tpp[:], rel_scan[:, base * C:(base + 2) * C],
                                        idf16[:])
                    nc.vector.tensor_copy(rlt[:, a::FSX], tpp[:])
                o2f = sp.tile([128, U], F32, tag="o2f")
                for (b0, bn) in ob_blocks:
                    ps = pcv.tile([128, 512], F32, tag="pconv")
                    nc.tensor.matmul(ps[:, 0:bn], g1[:], rlt[:, b0 + 1:b0 + 1 + bn],
                                     start=True, stop=False)
                    nc.tensor.matmul(ps[:, 0:bn], g2[:], rlt[:, b0 + 2:b0 + 2 + bn],
                                     start=False, stop=True)
                    nc.vector.tensor_copy(o2f[:, b0:b0 + bn], ps[:, 0:bn])
                oct_ = sp.tile([128, U], F32, tag="oct")
                for b in range(U // 128):
                    tpp = ptp.tile([128, 128], F32, tag="tp32")
                    nc.tensor.transpose(tpp[:], o2f[:, b::U // 128], idf32[:])
                    nc.scalar.activation(oct_[:, b * 128:(b + 1) * 128], tpp[:],
                                         Act.Identity,
                                         bias=FBT[:, c:c + 1], scale=FNT[:, c:c + 1])
                nc.sync.dma_start(out_e[c].rearrange("(p u) -> p u", u=U), oct_[:])

    nc.compile()
    prog = _Prog()
    prog.nc = nc
    _PROG_CACHE[key] = prog
    return prog


def host_prep(inputs):
    x = _f32(inputs["x"])
    # xs_k[u] = xp_full[512 + k*SP + u]
    need = 512 + (NCORES - 1) * SP + XS_LEN
    tail = need - (PADDING + len(x))
    xp_ext = np.concatenate([
        np.full(PADDING, x[0], np.float32), x,
        np.full(max(tail, 8), x[-1], np.float32)])

    w = _compute_kernel_np(inputs["log_kernel_speed"], inputs["cell_types"])
    W1 = np.zeros((NUM_CELLS, 128, 128), np.float16)
    W2 = np.zeros((NUM_CELLS, 128, 128), np.float16)
    for c in range(NUM_CELLS):
        for p in range(128):
            for m_ in range(128):
                d1 = p - m_
                if 0 <= d1 < K0:
                    W1[c, p, m_] = w[c, d1]
                d2 = 128 + p - m_
                if 0 <= d2 < K0:
                    W2[c, p, m_] = w[c, d2]
    g = _f32(inputs["iglusnfr_kernel"]).reshape(-1)
    G1 = np.zeros((128, 128), np.float16)
    G2 = np.zeros((128, 128), np.float16)
    for p in range(128):
        for m_ in range(128):
            d1 = p - m_
            if 0 <= d1 < K1:
                G1[p, m_] = g[d1]
            d2 = 128 + p - m_
            if 0 <= d2 < K1:
                G2[p, m_] = g[d2]

    Hr = np.exp(_smooth_clamp_np(_f32(inputs["log_release_pool_capacity"]), 1e6)).astype(np.float32)
    Hi = np.exp(_smooth_clamp_np(_f32(inputs["log_intermediate_pool_capacity"]), 1e6)).astype(np.float32)
    cp01 = np.exp(_f32(inputs["log_change_prob01"])).astype(np.float32)
    cp12 = np.exp(_f32(inputs["log_change_prob12"])).astype(np.float32)
    cc = np.zeros((8, FREE), np.float16)
    rows = [cp12, Hr, Hi, cp01, Hr - np.float32(EHARD), Hi - np.float32(EHARD),
            (Hr + 15.0) / 16.0 + 15.0 / 16.0, (Hi + 15.0) / 16.0 + 15.0 / 16.0]
    for i, v in enumerate(rows):
        cc[i] = np.repeat(v.astype(np.float32), FSLOT).astype(np.float16)
    slope = np.exp(_f32(inputs["log_sigmoid_slope"])).astype(np.float32)
    off = _f32(inputs["sigmoid_offset"])
    sg = np.stack([(-slope).astype(np.float32), (slope * off).astype(np.float32)])
    fs = np.exp(_f32(inputs["log_final_scale"])).astype(np.float32)
    fb = _f32(inputs["final_bias"])
    fn = np.stack([fs, fb])

    ip0 = (1.0 / (1.0 + np.exp(-_f32(inputs["sig_ip_steady"])))
           * np.exp(_f32(inputs["log_intermediate_pool_capacity"]))).astype(np.float32)
    rrp0 = (1.0 / (1.0 + np.exp(-_f32(inputs["sig_rrp_steady"])))
            * np.exp(_f32(inputs["log_release_pool_capacity"]))).astype(np.float32)

    params = dict(
        w1=W1, w2=W2, g1=G1, g2=G2,
        idf32=np.eye(128, dtype=np.float32),
        idf16=np.eye(128, dtype=np.float16),
        cc=cc, sg=sg, fn=fn)
    return xp_ext, params, rrp0, ip0


def make_inputs_for_core(k, xp_ext, params, rrp0, ip0):
    base = 512 + k * SP
    m = dict(params)
    m["xs"] = np.ascontiguousarray(xp_ext[base:base + XS_LEN]).astype(np.float16)
    return m


def kernel(**inputs):
    from concourse.bass_utils import run_bass_kernel_spmd
    prog = build_program()
    xp_ext, params, rrp0, ip0 = host_prep(inputs)
    in_maps = [make_inputs_for_core(k, xp_ext, params, rrp0, ip0)
               for k in range(NCORES)]
    res = run_bass_kernel_spmd(prog.nc, in_maps, list(range(NCORES)))
    out = np.concatenate([res.results[k]["out"] for k in range(NCORES)], axis=1)
    return out.astype(np.float32)
